# revision 17
# baseline (speedup 1.0000x reference)
"""Causal self-attention (GQA + RoPE) TP-sharded over 8 trn2 NeuronCores.

Sharding: core c owns Q heads {2c, 2c+1} and KV head c//2 (GQA rep=4 means
both Q heads map to the same KV head). Each core computes its head-shard of
q/k/v projections + rotary + causal attention + a partial o_proj against its
256-column shard of Wo. The host sums the 8 partial outputs.

Layouts (per core):
  xT   [2048, 4096]  x transposed (contraction dim on partitions)
  qT/kT [128, 2048]  per head, head_dim on partitions (scores contraction)
  v_nat [128, 16, 128] natural [t, d] chunks via PE transpose (PV contraction)
  scores kept transposed [tk, tq]: softmax denom via ones-matmul on PE,
  no max subtraction (weights are 0.02-scale, scores are O(1), exp is safe).
All matmul operands are float32r (single-pass fp22 multiply, fp32 accumulate).
"""

import sys

try:
    import concourse.bass as bass  # noqa: F401
except ImportError:
    sys.path.insert(0, "/opt/trn_rl_repo")

import math
from contextlib import ExitStack

import numpy as np

import concourse.bass as bass
import concourse.mybir as mybir
import concourse.tile as tile
from concourse import bacc
from concourse.bass_utils import run_bass_kernel_spmd

F32 = mybir.dt.float32
F32R = mybir.dt.float32r

B, T, C = 2, 2048, 2048
BT = B * T
N_HEAD, N_KV_HEAD, HD = 16, 4, 128
ROTARY_BASE = 10000
N_CORES = 8
QSH = 2 * HD  # q output dims per core (2 heads)
SCALE = 1.0 / math.sqrt(HD)

TT = 512  # t-tile (moving-operand free size)
NT = T // TT  # t tiles per batch (4)
KC = C // 128  # contraction chunks for projections (16)


def _sin_cos_np():
    # mirror reference._sin_cos bit-for-bit (float32 throughout)
    pos = np.arange(T, dtype=np.float32)
    dim = np.arange(HD // 2, dtype=np.float32)
    freq = (np.float32(ROTARY_BASE) ** (dim / np.float32(HD / 2))).astype(np.float32)
    freq = np.concatenate([freq, freq])
    angles = pos[:, None] / freq[None, :]
    return np.sin(angles).astype(np.float32), np.cos(angles).astype(np.float32)


def build_kernel():
    nc = bacc.Bacc()
    xT = nc.dram_tensor("xT", [C, BT], F32R, kind="ExternalInput")
    wq = nc.dram_tensor("wq", [C, QSH], F32R, kind="ExternalInput")
    wk = nc.dram_tensor("wk", [C, HD], F32R, kind="ExternalInput")
    wv = nc.dram_tensor("wv", [C, HD], F32R, kind="ExternalInput")
    wo = nc.dram_tensor("wo", [QSH, C], F32R, kind="ExternalInput")
    cosd = nc.dram_tensor("cosd", [HD, T], F32, kind="ExternalInput")
    sind = nc.dram_tensor("sind", [HD, T], F32, kind="ExternalInput")  # rot+signed
    trid = nc.dram_tensor("trid", [128, 128], F32R, kind="ExternalInput")
    identd = nc.dram_tensor("identd", [128, 128], F32R, kind="ExternalInput")
    onesd = nc.dram_tensor("onesd", [128, 1], F32R, kind="ExternalInput")
    out = nc.dram_tensor("out", [BT, C], F32, kind="ExternalOutput")

    with ExitStack() as ctx:
        tc = ctx.enter_context(tile.TileContext(nc))
        consts = ctx.enter_context(tc.tile_pool(name="consts", bufs=1))
        xpool = ctx.enter_context(tc.tile_pool(name="xc", bufs=18))
        qkpool = ctx.enter_context(tc.tile_pool(name="qk", bufs=8))
        kpool = ctx.enter_context(tc.tile_pool(name="kT", bufs=6))
        vpool = ctx.enter_context(tc.tile_pool(name="vnat", bufs=6))
        vtpool = ctx.enter_context(tc.tile_pool(name="vt", bufs=2))
        tmppool = ctx.enter_context(tc.tile_pool(name="ropetmp", bufs=3))
        ppool = ctx.enter_context(tc.tile_pool(name="pT", bufs=3))
        ytpool = ctx.enter_context(tc.tile_pool(name="yT", bufs=8))
        rcpool = ctx.enter_context(tc.tile_pool(name="rcp", bufs=3))
        rbcpool = ctx.enter_context(tc.tile_pool(name="rbc", bufs=2))
        outpool = ctx.enter_context(tc.tile_pool(name="osb", bufs=2))
        drampool = ctx.enter_context(
            tc.tile_pool(name="dscratch", bufs=4, space="DRAM")
        )

        # one dynamic psum pool: all 8 banks shared across phases
        ps = ctx.enter_context(tc.tile_pool(name="ps", bufs=8, space="PSUM"))

        def pstile(shape, dtype, name):
            return ps.tile(shape, dtype, tag="ps", name=name)

        # resident weights, loaded per 128-row chunk so consumers wait only on
        # their own slice; issued on the ACT queue (idle at startup) so the
        # sync queue services the x-chunk DMAs immediately
        wq_sb, wk_sb, wv_sb = [], [], []
        for kc in range(KC):
            r = slice(128 * kc, 128 * kc + 128)
            wq_sb.append(consts.tile([128, QSH], F32R, name=f"wq_{kc}"))
            nc.scalar.dma_start(out=wq_sb[kc], in_=wq.ap()[r, :])
            wk_sb.append(consts.tile([128, HD], F32R, name=f"wk_{kc}"))
            nc.scalar.dma_start(out=wk_sb[kc], in_=wk.ap()[r, :])
            wv_sb.append(consts.tile([128, HD], F32R, name=f"wv_{kc}"))
            nc.scalar.dma_start(out=wv_sb[kc], in_=wv.ap()[r, :])

        wo_sb = consts.tile([128, 2, C], F32R)
        cos_sb = consts.tile([HD, T], F32)
        sin_sb = consts.tile([HD, T], F32)
        tri_sb = consts.tile([128, 128], F32R)
        id_sb = consts.tile([128, 128], F32R)
        ones_sb = consts.tile([128, 1], F32R)

        def load_late_consts():
            # emitted after the first projection tile's matmuls; ACT queue
            nc.scalar.dma_start(out=cos_sb, in_=cosd.ap())
            nc.scalar.dma_start(out=sin_sb, in_=sind.ap())
            nc.scalar.dma_start(out=tri_sb, in_=trid.ap())
            nc.scalar.dma_start(out=id_sb, in_=identd.ap())
            nc.scalar.dma_start(out=ones_sb, in_=onesd.ap())
            nc.scalar.dma_start(
                out=wo_sb, in_=wo.ap().rearrange("(h p) n -> p h n", p=128)
            )

        xT_ap = xT.ap()
        out_ap = out.ap()

        def rope_evac(dst, pj, tpos):
            """dst = pj*cos + rotate_half(pj)*sin, psum -> sbuf.

            sind rows are pre-rotated by 64 and sign-folded on the host.
            """
            cs = cos_sb[:, tpos : tpos + TT]
            sn = sin_sb[:, tpos : tpos + TT]
            tmp = tmppool.tile([128, TT], F32)
            nc.vector.tensor_mul(tmp[0:64], pj[64:128], sn[64:128])
            nc.vector.tensor_mul(tmp[64:128], pj[0:64], sn[0:64])
            nc.vector.tensor_mul(dst, pj, cs)  # last psum read: frees the bank
            nc.vector.tensor_add(dst, dst, tmp)

        for b in range(B):
            # ---------------- projections for batch b ----------------
            qT = [
                [
                    qkpool.tile([128, TT], F32R, tag="qT", name=f"qT_{b}_{h}_{j}")
                    for j in range(NT)
                ]
                for h in range(2)
            ]
            kT = [
                kpool.tile([128, TT], F32R, tag="kT", name=f"kT_{b}_{j}")
                for j in range(NT)
            ]
            v_sb = [
                vpool.tile([128, 4, HD], F32R, tag="v", name=f"v_{b}_{j}")
                for j in range(NT)
            ]

            for jt in range(NT):
                tcol = b * T + jt * TT
                tpos = jt * TT
                xc = [
                    xpool.tile([128, TT], F32R, tag="xc", name=f"xc_{b}_{jt}_{kc}")
                    for kc in range(KC)
                ]
                for kc in range(KC):
                    nc.sync.dma_start(
                        out=xc[kc],
                        in_=xT_ap[128 * kc : 128 * kc + 128, tcol : tcol + TT],
                    )
                pq = [pstile([128, TT], F32, f"pq_{b}_{jt}_{h}") for h in range(2)]
                for kc in range(KC):
                    st, sp = (kc == 0), (kc == KC - 1)
                    for h in range(2):
                        nc.tensor.matmul(
                            pq[h],
                            wq_sb[kc][:, 128 * h : 128 * h + 128],
                            xc[kc],
                            start=st,
                            stop=sp,
                        )
                if b == 0 and jt == 0:
                    load_late_consts()
                rope_evac(qT[0][jt], pq[0], tpos)
                rope_evac(qT[1][jt], pq[1], tpos)
                pk = pstile([128, TT], F32, f"pk_{b}_{jt}")
                pv = pstile([128, TT], F32, f"pv_{b}_{jt}")
                for kc in range(KC):
                    st, sp = (kc == 0), (kc == KC - 1)
                    nc.tensor.matmul(pk, wk_sb[kc], xc[kc], start=st, stop=sp)
                    nc.tensor.matmul(pv, wv_sb[kc], xc[kc], start=st, stop=sp)
                rope_evac(kT[jt], pk, tpos)
                vt_sb = vtpool.tile([128, TT], F32R)
                nc.scalar.copy(vt_sb, pv)
                vt_ps = pstile([128, 4, 128], F32R, f"vtp_{b}_{jt}")
                for i in range(4):
                    nc.tensor.transpose(
                        vt_ps[:, i, :], vt_sb[:, 128 * i : 128 * i + 128], id_sb
                    )
                nc.vector.tensor_copy(v_sb[jt], vt_ps)

            # ---------------- attention for batch b ----------------
            yT = [
                [
                    ytpool.tile([128, TT], F32R, tag="yT", name=f"yT_{b}_{h}_{j}")
                    for j in range(NT)
                ]
                for h in range(2)
            ]
            for h in range(2):
                for j in range(NT):
                    if j == 0:
                        chunks = [(m, 128 * m) for m in (0, 1, 2, 3)]
                    else:
                        chunks = [(0, 0)]
                        chunks += [(4 * j + m, 128 * m) for m in (0, 1, 2, 3)]
                        chunks += [(c, 0) for c in range(1, 4 * j)]
                    nch = len(chunks)
                    yp = pstile([128, TT], F32, f"yp_{b}_{h}_{j}")
                    rp = pstile([1, TT], F32, f"rp_{b}_{h}_{j}")
                    for idx, (cch, off) in enumerate(chunks):
                        sT = pstile([128, TT], F32, f"sT_{b}_{h}_{j}_{idx}")
                        nc.tensor.matmul(
                            sT[:, off:],
                            kT[cch // 4][:, 128 * (cch % 4) : 128 * (cch % 4) + 128],
                            qT[h][j][:, off:],
                            start=True,
                            stop=True,
                        )
                        pT = ppool.tile([128, TT], F32R, tag="p")
                        nc.scalar.activation(
                            out=pT[:, off:],
                            in_=sT[:, off:],
                            func=mybir.ActivationFunctionType.Exp,
                            scale=SCALE,
                        )
                        if cch >= 4 * j:  # diagonal block: causal triangle
                            nc.vector.tensor_mul(
                                pT[:, off : off + 128],
                                pT[:, off : off + 128],
                                tri_sb,
                            )
                        nc.tensor.matmul(
                            yp[:, off:],
                            v_sb[cch // 4][:, cch % 4, :],
                            pT[:, off:],
                            start=(idx == 0),
                            stop=(idx == nch - 1),
                        )
                        nc.tensor.matmul(
                            rp[:, off:],
                            ones_sb,
                            pT[:, off:],
                            start=(idx == 0),
                            stop=(idx == nch - 1),
                        )
                    ysl = yT[h][j]
                    rcp = rcpool.tile(
                        [1, TT], F32, tag="rcp", name=f"rcp_{b}_{h}_{j}"
                    )
                    nc.vector.reciprocal(rcp, rp)  # frees the rowsum bank
                    nc.scalar.copy(ysl, yp)  # frees the PV bank
                    rdr = drampool.tile(
                        [1, TT], F32, tag="rdr", name=f"rdr_{b}_{h}_{j}"
                    )
                    nc.sync.dma_start(out=rdr, in_=rcp)
                    rbc = rbcpool.tile([128, TT], F32)
                    nc.sync.dma_start(
                        out=rbc,
                        in_=bass.AP(
                            tensor=rdr.tensor,
                            offset=rdr.offset,
                            ap=[[0, 128], rdr.ap[-1]],
                        ),
                    )
                    nc.gpsimd.tensor_mul(ysl, ysl, rbc)  # off the DVE queue

            # ---------------- partial o_proj for batch b ----------------
            for ts_ in range(T // 128):
                row = b * T + 128 * ts_
                osb = outpool.tile([128, C], F32, tag="osb", name=f"osb_{b}_{ts_}")
                for n in range(C // TT):
                    op = pstile([128, TT], F32, f"op_{b}_{ts_}_{n}")
                    for h in range(2):
                        nc.tensor.matmul(
                            op,
                            yT[h][ts_ // 4][:, 128 * (ts_ % 4) : 128 * (ts_ % 4) + 128],
                            wo_sb[:, h, TT * n : TT * n + TT],
                            start=(h == 0),
                            stop=(h == 1),
                        )
                    if n % 2 == 0:
                        nc.scalar.copy(osb[:, TT * n : TT * n + TT], op)
                    else:
                        nc.vector.tensor_copy(osb[:, TT * n : TT * n + TT], op)
                nc.sync.dma_start(out=out_ap[row : row + 128, :], in_=osb)

    nc.finalize()
    return nc


_NC_CACHE = None
TRACE = False
LAST_RESULTS = None


def _get_nc():
    global _NC_CACHE
    if _NC_CACHE is None:
        _NC_CACHE = build_kernel()
    return _NC_CACHE


def kernel(x, Wq, Wk, Wv, Wo):
    x = np.asarray(x, dtype=np.float32)
    Wq = np.asarray(Wq, dtype=np.float32)
    Wk = np.asarray(Wk, dtype=np.float32)
    Wv = np.asarray(Wv, dtype=np.float32)
    Wo = np.asarray(Wo, dtype=np.float32)

    xT = np.ascontiguousarray(x.reshape(BT, C).T)
    sin_, cos_ = _sin_cos_np()  # [T, 128]
    cosd = np.ascontiguousarray(cos_.T)
    sinT = np.ascontiguousarray(sin_.T)
    # row-rotated by 64 and sign-folded: output rows 0:64 read input rows
    # 64:128 (value -sin), output rows 64:128 read input rows 0:64 (+sin)
    sind = np.empty_like(sinT)
    sind[64:128] = -sinT[0:64]
    sind[0:64] = sinT[64:128]
    trid = np.triu(np.ones((128, 128), dtype=np.float32))
    identd = np.eye(128, dtype=np.float32)
    onesd = np.ones((128, 1), dtype=np.float32)

    core_ids = list(range(N_CORES))
    in_maps = []
    for c in core_ids:
        g = c // 2
        in_maps.append(
            {
                "xT": xT,
                "wq": np.ascontiguousarray(Wq[QSH * c : QSH * (c + 1)].T),
                "wk": np.ascontiguousarray(Wk[HD * g : HD * (g + 1)].T),
                "wv": np.ascontiguousarray(Wv[HD * g : HD * (g + 1)].T),
                "wo": np.ascontiguousarray(Wo[:, QSH * c : QSH * (c + 1)].T),
                "cosd": cosd,
                "sind": sind,
                "trid": trid,
                "identd": identd,
                "onesd": onesd,
            }
        )
    global LAST_RESULTS
    res = run_bass_kernel_spmd(_get_nc(), in_maps, core_ids, trace=TRACE)
    LAST_RESULTS = res
    total = res.results[0]["out"].astype(np.float32)
    for c in core_ids[1:]:
        total = total + res.results[c]["out"]
    return total.reshape(B, T, C)
